# revision 4
# baseline (speedup 1.0000x reference)
"""Distributed Trainium2 Bass kernel for a full attention layer (prefill).

Reference computation (B=4, S=1024, D=4096, H=32, HD=128, fp32 I/O):
    xq = rope(x @ wq.T), xk = rope(x @ wk.T), xv = x @ wv.T
    out = softmax(causal(xq xk^T / sqrt(HD))) @ xv
    y   = out @ wo.T

Sharding: 8-way tensor parallel over heads (4 heads / core).

v2 structure (vs v1): per-batch fusion.  For each batch b the core
projects the batch's two 512-token chunks (q/k RoPE'd straight into
SBUF, v token-major in SBUF — no DRAM spill), then runs attention on
its 4 heads while the next batch's x streams in.  The AllGather of
per-head attention outputs is issued at the end of A(b) and is hidden
under later batches' compute; the wo phases run last, so even the
final batch's AllGather hides under the other batches' wo matmuls.

Layout notes (everything "transposed", i.e. feature-major):
  - x is fed as xT [D, B*S] so projections produce qT/kT [o, tok]
    directly and the attention chain needs no on-chip transposes:
      scoresT[j,i] = kT_tile^T @ qT_chunk        (j keys, i queries)
      softmax over j (partition axis) via exp + ones-matmul column sums
      attn_T[hd,i] = v_tile^T(nat) @ probsT      (v kept token-major)
  - RoPE pairs are split (re | im halves) per head by permuting
    wq/wk rows on the host.  The cross-partition half-swap is done with
    two PSUM->SBUF DMAs, then the rotation is 3 full-width DVE ops:
      new = q * [c;c] + swap(q) * [-s;s]
  - Causal mask: block-skip fully-masked (j,i) tiles; the only nonzero
    mask piece is the 128x128 upper-triangular band on the diagonal
    block (identical for every diagonal tile).
  - Softmax skips max-subtraction: scores ~ N(0,1), exp safe in fp32.
  - Attention emission interleaves scores(h) with pv(h-1) per j-tile so
    the PE never stalls on the Scalar engine's exp drain; su (column
    sums) run as one stationary-shared ones-matmul group per head.
"""

import math
import os
import sys

import numpy as np

for _p in ("/opt/trn_rl_repo", "/root/.axon_site/_ro/trn_rl_repo"):
    if os.path.isdir(_p) and _p not in sys.path:
        sys.path.insert(0, _p)

import ml_dtypes  # noqa: E402
import concourse.bass as bass  # noqa: E402
import concourse.mybir as mybir  # noqa: E402
import concourse.tile as tile  # noqa: E402
from concourse import bacc  # noqa: E402
from concourse.bass_utils import run_bass_kernel_spmd  # noqa: E402

B, S, D, H = 4, 1024, 4096, 32
HD = D // H            # 128
NC = 8                 # cores
HPC = H // NC          # 4 heads per core
OC = HPC * HD          # 512 output dims per core
NT = B * S             # 4096 tokens
P = 128
KT = D // P            # 32 contraction tiles
KP = 8                 # k-parts per chunk DMA (for startup pipelining)
KTP = KT // KP         # 4 k-tiles per part
TCH = 512              # token chunk (columns per projection matmul)
NCH = NT // TCH        # 8 chunks
NJT = S // P           # 8 key tiles per batch
SCALE = 1.0 / math.sqrt(HD)

BF16 = mybir.dt.bfloat16
F32 = mybir.dt.float32


def build():
    nc = bacc.Bacc("TRN2", target_bir_lowering=False, debug=False,
                   num_devices=NC)

    # ---- I/O ----
    xT_d = nc.dram_tensor("xT", [NCH, P, KT, TCH], BF16,
                          kind="ExternalInput")
    wqT_d = nc.dram_tensor("wqT", [P, KT, OC], BF16, kind="ExternalInput")
    wkT_d = nc.dram_tensor("wkT", [P, KT, OC], BF16, kind="ExternalInput")
    wvT_d = nc.dram_tensor("wvT", [P, KT, OC], BF16, kind="ExternalInput")
    woT_d = nc.dram_tensor("woT", [P, KT, OC], BF16, kind="ExternalInput")
    ccT_d = nc.dram_tensor("ccT", [P, S], BF16, kind="ExternalInput")
    ssT_d = nc.dram_tensor("ssT", [P, S], BF16, kind="ExternalInput")
    mb_d = nc.dram_tensor("mband", [P, P], BF16, kind="ExternalInput")
    out_d = nc.dram_tensor("out", [OC, NT], F32, kind="ExternalOutput")

    # ---- internal DRAM (collectives only) ----
    # agin is pre-tiled [P, chunk, head, TCH] so the W phase reads each
    # core-block of agout as one contiguous 4KB-per-partition descriptor
    agin = [nc.dram_tensor(f"agin{b}", [P, 2, HPC, TCH], BF16)
            for b in range(B)]
    agout = [nc.dram_tensor(f"agout{b}", [NC, P, 2, HPC, TCH], BF16,
                            addr_space="Shared") for b in range(B)]

    def wpart(dram_ap, kp):
        """k-part kp of a pre-tiled [P, KT, n] weight tensor."""
        return dram_ap[:, kp * KTP:(kp + 1) * KTP, :]

    with tile.TileContext(nc) as tc, \
         tc.tile_pool(name="const", bufs=1) as cpool, \
         tc.tile_pool(name="pq", bufs=1) as pq, \
         tc.tile_pool(name="pp", bufs=2) as ppool, \
         tc.tile_pool(name="ao", bufs=2) as ao, \
         tc.tile_pool(name="asm", bufs=1) as asm, \
         tc.tile_pool(name="pps", bufs=4, space="PSUM") as pps:
        ccT = cpool.tile([P, S], BF16, tag="cc")
        ssT = cpool.tile([P, S], BF16, tag="ss")
        mband = cpool.tile([P, P], BF16, tag="mb")
        ones_col = cpool.tile([P, 1], BF16, tag="oc")
        nc.gpsimd.dma_start(ccT[:], ccT_d.ap())
        nc.gpsimd.dma_start(ssT[:], ssT_d.ap())
        nc.gpsimd.dma_start(mband[:], mb_d.ap())
        nc.vector.memset(ones_col[:], 1.0)

        with tc.tile_pool(name="pw", bufs=1) as pw, \
             tc.tile_pool(name="px", bufs=9) as px, \
             tc.tile_pool(name="pr", bufs=2) as pr, \
             tc.tile_pool(name="apv", bufs=1, space="PSUM") as apv, \
             tc.tile_pool(name="asu", bufs=1, space="PSUM") as asu:

            # persistent per-batch q/k/v activation tiles (single-buffered:
            # PE program order guarantees A(b) reads precede P(b+1) writes)
            q_sb = pq.tile([P, HPC, S], BF16, tag="q")
            k_sb = pq.tile([P, HPC, S], BF16, tag="k")
            v_sb = pq.tile([P, NJT, OC], BF16, tag="v")

            def load_x(ch, rings=(nc.scalar,)):
                xc = [px.tile([P, KTP, TCH], BF16, tag="x",
                              name=f"xc{ch}_{kp}") for kp in range(KP)]
                for kp in range(KP):
                    rings[kp % len(rings)].dma_start(
                        xc[kp][:],
                        xT_d.ap()[ch, :, kp * KTP:(kp + 1) * KTP, :])
                return xc

            # chunk 0's x streams ahead of the weight queue
            xc0 = load_x(0, rings=(nc.scalar, nc.gpsimd))

            # resident qkv weights; wq part0 first so matmuls start early
            w_sb = {}
            order = ([("q", kp) for kp in range(KP)]
                     + [("k", kp) for kp in range(KP)]
                     + [("v", kp) for kp in range(KP)])
            for wname, kp in order:
                wd = {"q": wqT_d, "k": wkT_d, "v": wvT_d}[wname]
                t = pw.tile([P, KTP, OC], BF16, tag=f"w{wname}{kp}")
                ring = {"q": nc.sync, "k": nc.scalar,
                        "v": nc.gpsimd}[wname]
                ring.dma_start(t[:], wpart(wd.ap(), kp))
                w_sb[(wname, kp)] = t

            # warm the exp activation-table set before attention needs it
            warm = cpool.tile([1, 2], F32, tag="warm")
            nc.vector.memset(warm[:], 0.0)
            nc.scalar.activation(warm[:], warm[:],
                                 mybir.ActivationFunctionType.Exp)

            # PE warm-up: dependency-free matmuls on memset data fill the
            # DMA cold-start dead zone and open the HAM clock gate (K=8/8
            # needs ~3.4us of sustained PE activity) before the first real
            # matmul's operands arrive
            dummy = cpool.tile([P, TCH], BF16, tag="dummy")
            nc.vector.memset(dummy[:], 0.0)
            dps = pps.tile([P, TCH], F32, tag="s", name="warmps")
            for i in range(40):
                nc.tensor.matmul(dps[:], lhsT=dummy[:, 0:P], rhs=dummy[:],
                                 start=(i == 0), stop=(i == 39))
            dsink = cpool.tile([P, 1], F32, tag="dsink")
            nc.vector.tensor_copy(dsink[:], dps[:, 0:1])

            # ---------- P: project one 512-token chunk ----------
            def proj_chunk(ch, between_qk_v=None, xc=None):
                cl = ch % 2              # chunk-in-batch
                psl = slice(cl * TCH, (cl + 1) * TCH)
                if xc is None:
                    xc = load_x(ch)

                def rope(ps, dst, ot):
                    # cross-partition half swap via bf16 SBUF bounce
                    qb = pr.tile([P, TCH], BF16, tag="qb")
                    nc.vector.tensor_copy(qb[:], ps[:])
                    sw = pr.tile([P, TCH], BF16, tag="sw")
                    nc.sync.dma_start(sw[0:64, :], qb[64:128, :])
                    nc.sync.dma_start(sw[64:128, :], qb[0:64, :])
                    qc = pr.tile([P, TCH], BF16, tag="qc")
                    qs = pr.tile([P, TCH], BF16, tag="qs")
                    nc.vector.tensor_tensor(
                        out=qc[:], in0=ps[:], in1=ccT[:, psl],
                        op=mybir.AluOpType.mult)
                    nc.vector.tensor_tensor(
                        out=qs[:], in0=sw[:], in1=ssT[:, psl],
                        op=mybir.AluOpType.mult)
                    nc.vector.tensor_tensor(
                        out=dst[:, ot, cl * TCH:(cl + 1) * TCH],
                        in0=qc[:], in1=qs[:],
                        op=mybir.AluOpType.add)

                # q/k projections (feature-major out) + RoPE into q_sb/k_sb
                for wname, dst in (("q", q_sb), ("k", k_sb)):
                    if ch > 0:
                        for ot in range(HPC):
                            osl = slice(ot * P, (ot + 1) * P)
                            ps = pps.tile([P, TCH], F32, tag="s")
                            for k in range(KT):
                                kp, ki = divmod(k, KTP)
                                nc.tensor.matmul(
                                    ps[:], lhsT=w_sb[(wname, kp)][:, ki, osl],
                                    rhs=xc[kp][:, ki, :],
                                    start=(k == 0), stop=(k == KT - 1))
                            rope(ps, dst, ot)
                    else:
                        # startup: the weight k-parts are still streaming in,
                        # so sweep all four output groups over the first half
                        # of k before the second half -- the PE advances on
                        # parts 0-3 while parts 4-7 arrive
                        tiles = [pps.tile([P, TCH], F32, tag="s",
                                          name=f"s0_{wname}{ot}")
                                 for ot in range(HPC)]
                        for half in range(2):
                            for ot in range(HPC):
                                osl = slice(ot * P, (ot + 1) * P)
                                for k in range(half * (KT // 2),
                                               (half + 1) * (KT // 2)):
                                    kp, ki = divmod(k, KTP)
                                    nc.tensor.matmul(
                                        tiles[ot][:],
                                        lhsT=w_sb[(wname, kp)][:, ki, osl],
                                        rhs=xc[kp][:, ki, :],
                                        start=(k == 0), stop=(k == KT - 1))
                                if half == 1:
                                    rope(tiles[ot], dst, ot)

                if between_qk_v is not None:
                    between_qk_v()

                # v projection (token-major out) into v_sb
                for jt in range(TCH // P):
                    jsl = slice(jt * P, (jt + 1) * P)
                    ps = pps.tile([P, OC], F32, tag="s")
                    for k in range(KT):
                        kp, ki = divmod(k, KTP)
                        nc.tensor.matmul(
                            ps[:], lhsT=xc[kp][:, ki, jsl],
                            rhs=w_sb[("v", kp)][:, ki, :],
                            start=(k == 0), stop=(k == KT - 1))
                    nc.vector.tensor_copy(v_sb[:, cl * 4 + jt, :], ps[:])

            # ---------- A: attention for batch b ----------
            def jmax(ic):       # causal: j tiles 0..jmax-1 for i-chunk ic
                return 4 * (ic + 1)

            def mslot(ic, jt):  # probsT slot index
                return jt if ic == 0 else 4 + jt

            def emit_scores_jt(h, jt, pp):
                """scores + exp for one key tile (both i-chunks), sharing
                the kT stationary between the two matmuls."""
                kTh = k_sb[:, h, jt * P:(jt + 1) * P]
                qTh = q_sb[:, h, :]
                for ic in (1, 0) if jt < 4 else (1,):
                    r = jt - 4 * ic
                    m = mslot(ic, jt)
                    if r < 0:
                        sps = pps.tile([P, TCH], F32, tag="s")
                        nc.tensor.matmul(
                            sps[:], lhsT=kTh,
                            rhs=qTh[:, ic * TCH:(ic + 1) * TCH],
                            start=True, stop=True)
                        nc.scalar.activation(
                            pp[:, m, :], sps[:],
                            mybir.ActivationFunctionType.Exp,
                            scale=SCALE)
                    else:
                        w = TCH - r * P   # live columns
                        sps = pps.tile([P, TCH], F32, tag="s")
                        nc.tensor.matmul(
                            sps[:, :w], lhsT=kTh,
                            rhs=qTh[:, ic * TCH + r * P:(ic + 1) * TCH],
                            start=True, stop=True)
                        nc.vector.tensor_tensor(
                            out=sps[:, 0:P], in0=sps[:, 0:P],
                            in1=mband[:],
                            op=mybir.AluOpType.add)
                        nc.scalar.activation(
                            pp[:, m, r * P:], sps[:, :w],
                            mybir.ActivationFunctionType.Exp,
                            scale=SCALE)

            def live(ic, jt):
                r = jt - 4 * ic
                return 0 if r <= 0 else r * P

            def emit_pv_jt(h, jt, pp, pv0, pv1):
                """pv accumulation for one key tile (both i-chunks),
                sharing the v stationary between the two matmuls."""
                vh = v_sb[:, jt, h * P:(h + 1) * P]
                for ic, pv in ((0, pv0), (1, pv1)):
                    if jt >= jmax(ic):
                        continue
                    o = live(ic, jt)
                    nc.tensor.matmul(
                        pv[:, o:], lhsT=vh,
                        rhs=pp[:, mslot(ic, jt), o:],
                        start=(jt == 0), stop=(jt == jmax(ic) - 1))

            def emit_su_norm(b, h, pp, pv0, pv1):
                """column sums (ones-matmul, one stationary), reciprocal,
                normalize, store to agin."""
                su0 = asu.tile([1, TCH], F32, tag="su0")
                su1 = asu.tile([1, TCH], F32, tag="su1")
                for ic, su in ((0, su0), (1, su1)):
                    jm = jmax(ic)
                    for jt in range(jm):
                        o = live(ic, jt)
                        nc.tensor.matmul(
                            su[:, o:], lhsT=ones_col[:],
                            rhs=pp[:, mslot(ic, jt), o:],
                            start=(jt == 0), stop=(jt == jm - 1))
                at = ao.tile([P, S], BF16, tag="at")
                for ic, su, pv in ((0, su0, pv0), (1, su1, pv1)):
                    rec = asm.tile([1, TCH], F32, tag="rec")
                    nc.vector.reciprocal_approx_fast(rec[:], su[:])
                    bcast = asm.tile([P, TCH], F32, tag="bcast")
                    nc.gpsimd.partition_broadcast(bcast[:], rec[:])
                    nc.vector.tensor_tensor(
                        out=at[:, ic * TCH:(ic + 1) * TCH], in0=pv[:],
                        in1=bcast[:], op=mybir.AluOpType.mult)
                for c2 in range(2):
                    nc.sync.dma_start(
                        agin[b].ap()[:, c2, h, :],
                        at[:, c2 * TCH:(c2 + 1) * TCH])
                if h == HPC - 1:
                    nc.gpsimd.collective_compute(
                        "AllGather", mybir.AluOpType.bypass,
                        ins=[agin[b].ap().opt()],
                        outs=[agout[b].ap().opt()],
                        replica_groups=[list(range(NC))])

            def do_attn(b, pp_h0):
                pv0_h0 = apv.tile([P, TCH], F32, tag="pv0", name="pv0_h0")
                pv1_h0 = apv.tile([P, TCH], F32, tag="pv1", name="pv1_h0")
                prev = (pp_h0, pv0_h0, pv1_h0)
                for h in range(1, HPC):
                    pp = ppool.tile([P, 12, TCH], BF16, tag="pp")
                    pv0 = apv.tile([P, TCH], F32, tag="pv0")
                    pv1 = apv.tile([P, TCH], F32, tag="pv1")
                    for jt in range(NJT):
                        emit_scores_jt(h, jt, pp)
                        emit_pv_jt(h - 1, jt, *prev)
                    emit_su_norm(b, h - 1, *prev)
                    prev = (pp, pv0, pv1)
                for jt in range(NJT):
                    emit_pv_jt(HPC - 1, jt, *prev)
                emit_su_norm(b, HPC - 1, *prev)

            for b in range(B):
                proj_chunk(2 * b, xc=(xc0 if b == 0 else None))
                # head 0's scores are emitted between the q/k and v
                # projections of the second chunk so its exps drain on the
                # Scalar engine while the PE runs the v projection -- the
                # attention pv pipeline then starts with no fill stall
                state = {}

                def scores_h0():
                    pp = ppool.tile([P, 12, TCH], BF16, tag="pp",
                                    name="pp_h0")
                    for jt in range(NJT):
                        emit_scores_jt(0, jt, pp)
                    state["pp0"] = pp

                proj_chunk(2 * b + 1, between_qk_v=scores_h0)
                do_attn(b, state["pp0"])

        # ---------- W: output projection, all batches ----------
        with tc.tile_pool(name="ww", bufs=1) as ww, \
             tc.tile_pool(name="wg", bufs=24) as wg, \
             tc.tile_pool(name="wy", bufs=4) as wy:
            wps = pps

            # wo on the gpsimd ring (idle, not queued behind attention
            # exps); k-part tiles so the first W matmuls start as soon as
            # part 0 lands rather than after the whole 4MB
            wo_sb = {}
            for kp in range(KP):
                t = ww.tile([P, KTP, OC], BF16, tag=f"wo{kp}")
                nc.gpsimd.dma_start(t[:], wpart(woT_d.ap(), kp))
                wo_sb[kp] = t

            def do_wo(b):
                # both chunks of the batch resident: the wo stationary is
                # shared between the two chunks' matmuls (paired C=2).
                # agc part kp is source-core kp's block of the AllGather:
                # one contiguous 4KB-per-partition descriptor.
                agc = [[wg.tile([P, HPC, TCH], BF16, tag="ag",
                                name=f"agc{c}_{kp}") for kp in range(KP)]
                       for c in range(2)]
                for kp in range(KP):
                    ring = (nc.sync, nc.scalar)[kp % 2]
                    for c in range(2):
                        ring.dma_start(agc[c][kp][:],
                                       agout[b].ap()[kp, :, c, :, :])
                for ot in range(HPC):
                    osl = slice(ot * P, (ot + 1) * P)
                    ps = [wps.tile([P, TCH], F32, tag="s", name=f"y{c}")
                          for c in range(2)]
                    for k in range(KT):
                        kp, ki = divmod(k, KTP)
                        for c in range(2):
                            nc.tensor.matmul(
                                ps[c][:], lhsT=wo_sb[kp][:, ki, osl],
                                rhs=agc[c][kp][:, ki, :],
                                start=(k == 0), stop=(k == KT - 1))
                    for c in range(2):
                        ch = b * 2 + c
                        yt = wy.tile([P, TCH], F32, tag="y")
                        nc.vector.tensor_copy(yt[:], ps[c][:])
                        nc.scalar.dma_start(
                            out_d.ap()[osl, ch * TCH:(ch + 1) * TCH], yt[:])

            for b in range(B):
                do_wo(b)

    nc.compile()
    return nc


_BUILT = {}


def _get_nc():
    if "nc" not in _BUILT:
        _BUILT["nc"] = build()
    return _BUILT["nc"]


def _tile_w(w_slice):
    """[OC, D] weight slice -> pre-tiled lhsT image [P, KT, OC] bf16."""
    return np.ascontiguousarray(
        w_slice.T.reshape(KT, P, OC).transpose(1, 0, 2)
        .astype(ml_dtypes.bfloat16))


def _prep_inputs(x, wq, wk, wv, wo, freqs_cos, freqs_sin, mask):
    bf = ml_dtypes.bfloat16
    # x -> [NCH, P, KT, TCH] with xtc[ch, p, k, n] = x[512ch+n, 128k+p]
    xT = np.ascontiguousarray(
        np.asarray(x).reshape(NCH, TCH, KT, P).transpose(0, 3, 2, 1)
        .astype(bf))

    # split-halves RoPE permutation of q/k rows, per head
    perm = np.concatenate([np.arange(0, HD, 2), np.arange(1, HD, 2)])
    full_perm = (np.arange(H)[:, None] * HD + perm[None, :]).reshape(-1)
    wq_p = np.asarray(wq)[full_perm]
    wk_p = np.asarray(wk)[full_perm]

    ccT = np.empty((P, S), ml_dtypes.bfloat16)
    ssT = np.empty((P, S), ml_dtypes.bfloat16)
    ct = np.asarray(freqs_cos).T          # [64, S]
    st = np.asarray(freqs_sin).T
    ccT[0:64], ccT[64:128] = ct, ct
    ssT[0:64], ssT[64:128] = -st, st      # new = q*[c;c] + swap(q)*[-s;s]

    m2 = np.asarray(mask)[0, 0]           # [S, S], mask[i, j]
    # the only nonzero mask piece: 128x128 diagonal block, band[jl, il] =
    # mask[il, jl] (identical for every diagonal tile)
    mband = np.ascontiguousarray(m2[0:P, 0:P].T.astype(ml_dtypes.bfloat16))

    in_maps = []
    for c in range(NC):
        osl = slice(c * OC, (c + 1) * OC)
        in_maps.append({
            "xT": xT,
            "wqT": _tile_w(wq_p[osl]),
            "wkT": _tile_w(wk_p[osl]),
            "wvT": _tile_w(np.asarray(wv)[osl]),
            "woT": _tile_w(np.asarray(wo)[osl]),
            "ccT": ccT,
            "ssT": ssT,
            "mband": mband,
        })
    return in_maps


def kernel(x, wq, wk, wv, wo, freqs_cos, freqs_sin, mask, _results_out=None):
    nc = _get_nc()
    in_maps = _prep_inputs(x, wq, wk, wv, wo, freqs_cos, freqs_sin, mask)
    res = run_bass_kernel_spmd(nc, in_maps, core_ids=list(range(NC)))
    if _results_out is not None:
        _results_out.append(res)
    yT = np.concatenate([res.results[c]["out"] for c in range(NC)], axis=0)
    return np.ascontiguousarray(yT.T).reshape(B, S, D).astype(np.float32)
